# revision 1
# baseline (speedup 1.0000x reference)
"""Equivariant layer block (order-2, 15-basis) on 8 Trainium2 NeuronCores.

Decomposition (indices: c in-channel, o out-channel, n/m spatial, N=2048):
  Y[o,n,m] = sum_c X[c,n,m] W8[c,o] + X[c,m,n] W6[c,o]
           + A[o,n] + B[o,m] + D[o,n] delta[n,m]
with (raw sums; /N factors folded into host-side weights; i = ref basis index)
  A[o,n] = dv.W5 + csum.W7/N + rsum.W12/N + dsum.W11/N + tsum.W14/N^2 + sum(bias)
  B[o,m] = dv.W9 + csum.W10/N + rsum.W13/N
  D[o,n] = dv.W0 + csum.W1/N + rsum.W3/N + dsum.W2/N + tsum.W4/N^2

v2 design. Core k owns output rows I_k=[256k,256k+256). Both spatial panels are
packed fp8e4 into ONE SBUF tile rc[(g,c), kt, n', m_w] (kt=0 row panel, kt=1
col panel), so the main loop does a single fp8 DoubleRow matmul per 2-row
slice (identity + transpose terms fused via the two k-tiles at 0.5 cyc/row).
The per-column table B is added on the PE too, via a one-hot DoubleRow matmul
whose stationary operand is the fp8 transpose of the B table; A is added by
the vector engine during PSUM eviction (broadcast along m); the diagonal D
fix-up is a tiny strided vector add. Output is written fp16 (host upcasts).

Stats: each chunk is upconverted fp8->fp16 (scalar/gpsimd engines, otherwise
idle), the row sums come from a DVE halving tree, and the column sums from
PE transposes that accumulate into a persistent fp16 PSUM tile (the tensor
engine is idle during the load phase). One fp16 AllReduce of [128, 513]
(pre-folded B table | packed csum partials | masked diag column) globalizes
the column stats. PSUM pools for stats and main loop are disjoint so the main
loop never serializes behind the post-collective chain.
"""

import os
import numpy as np

import concourse.bacc as bacc
import concourse.tile as tile
import concourse.mybir as mybir
from concourse import bass_utils

N = 2048
C = 16
NCORES = 8
RPC = N // NCORES  # 256 rows per core
G = 8  # m-groups
MW = N // G  # 256
P = 128
CHUNK = 16  # rows per DMA chunk
NCHUNK = RPC // CHUNK  # 16
QR = 4  # rows per main-loop quarter-chunk
NQ = RPC // QR  # 64
f16 = mybir.dt.float16
f32 = mybir.dt.float32
f8 = mybir.dt.float8e4

LAST_RUN_INFO = {}
_CACHED = {}


def _install_trace_hook():
    """Best-effort NTFF hook injection (used only when BASS_TRACE is set)."""
    try:
        import sys, types

        if "antenv.axon_hooks" in sys.modules:
            return
        mod = types.ModuleType("antenv.axon_hooks")
        state = {}
        mod.set_axon_ntff_profile_hook = lambda h: state.update(h=h)
        mod.get_axon_ntff_profile_hook = lambda: state.get("h")
        sys.modules["antenv.axon_hooks"] = mod
        import antenv

        antenv.axon_hooks = mod
        from trn_agent_boot.trn_boot import _ntff_profile_via_ctypes

        mod.set_axon_ntff_profile_hook(
            _ntff_profile_via_ctypes("/opt/axon/libaxon_pjrt.so")
        )
    except Exception:
        pass


def _build_program():
    nc = bacc.Bacc("TRN2", target_bir_lowering=False, debug=False, num_devices=NCORES)

    # interleaved panel: rc_d[p, b, t, r2, m], row = 2b+r2, t=0 rows / t=1 cols
    rc_d = nc.dram_tensor("rc8", [P, RPC // 2, 2, 2, MW], f8, kind="ExternalInput").ap()
    wst_d = nc.dram_tensor("wst", [P, 2, P], f8, kind="ExternalInput").ap()
    ohb_d = nc.dram_tensor("ohb", [P, 2, 2, MW], f8, kind="ExternalInput").ap()
    id8_d = nc.dram_tensor("id8", [P, 2, P], f8, kind="ExternalInput").ap()
    idf_d = nc.dram_tensor("idf", [P, P], f32, kind="ExternalInput").ap()
    wbcs_d = nc.dram_tensor("wb_cs", [P, P], f32, kind="ExternalInput").ap()
    wbdv_d = nc.dram_tensor("wb_dv", [P, P], f16, kind="ExternalInput").ap()
    wbrs_d = nc.dram_tensor("wb_rs", [P, P], f32, kind="ExternalInput").ap()
    gk16_d = nc.dram_tensor("gk16", [P, C], f16, kind="ExternalInput").ap()
    gk32_d = nc.dram_tensor("gk32", [P, C], f32, kind="ExternalInput").ap()
    gall_d = nc.dram_tensor("g_all", [P, C], f32, kind="ExternalInput").ap()
    wad_d = nc.dram_tensor("wad", [2, 96, P], f32, kind="ExternalInput").ap()
    wcc_d = nc.dram_tensor("wcc", [2, 48, P], f32, kind="ExternalInput").ap()
    smask_d = nc.dram_tensor("smask", [P, 1], f32, kind="ExternalInput").ap()
    bsum_d = nc.dram_tensor("bsum", [P, 1], f32, kind="ExternalInput").ap()

    y_d = nc.dram_tensor("y", [P, RPC, MW], f16, kind="ExternalOutput").ap()

    add = mybir.AluOpType.add
    COPY = mybir.ActivationFunctionType.Copy
    IDENT = mybir.ActivationFunctionType.Identity

    with tile.TileContext(nc) as tc:
        with (
            tc.tile_pool(name="small", bufs=1) as small,
            tc.tile_pool(name="rcp", bufs=1) as rcp,
            tc.tile_pool(name="treep", bufs=1) as treep,
            tc.tile_pool(name="stagep", bufs=4) as stagep,
            tc.tile_pool(name="psstat", bufs=1, space="PSUM") as psstat,
            tc.tile_pool(name="pscst", bufs=1, space="PSUM") as pscst,
            tc.tile_pool(name="psmain", bufs=3, space="PSUM") as psmain,
            tc.tile_pool(name="dram", bufs=1, space="DRAM") as dram,
        ):
            # ---- constant / weight loads ----
            wst = small.tile([P, 2, P], f8)
            ohb = small.tile([P, 2, 2, MW], f8)
            id8 = small.tile([P, 2, P], f8)
            idf = small.tile([P, P], f32)
            wb_cs = small.tile([P, P], f32)
            wb_dv = small.tile([P, P], f16)
            wb_rs = small.tile([P, P], f32)
            gk16 = small.tile([P, C], f16)
            gk32 = small.tile([P, C], f32)
            g_all = small.tile([P, C], f32)
            smask = small.tile([P, 1], f32)
            bsum = small.tile([P, 1], f32)
            for t, d in [
                (wst, wst_d),
                (ohb, ohb_d),
                (id8, id8_d),
                (idf, idf_d),
                (wb_cs, wbcs_d),
                (wb_dv, wbdv_d),
                (wb_rs, wbrs_d),
                (gk16, gk16_d),
                (gk32, gk32_d),
                (g_all, gall_d),
                (smask, smask_d),
                (bsum, bsum_d),
            ]:
                nc.sync.dma_start(t[:], d[:])
            wa2 = small.tile([96, P], f32)
            wd2 = small.tile([96, P], f32)
            wca2 = small.tile([48, P], f32)
            wcd2 = small.tile([48, P], f32)
            nc.sync.dma_start(wa2[:], wad_d[0])
            nc.sync.dma_start(wd2[:], wad_d[1])
            nc.sync.dma_start(wca2[:], wcc_d[0])
            nc.sync.dma_start(wcd2[:], wcc_d[1])

            # ---- resident fp8 panel, row-pair interleaved so the DoubleRow
            # k-tile stride (512 elem) fits the 16-bit ISA stride field ----
            # rc[p, b, t, r2, m]: row = 2*b + r2, t=0 row panel / t=1 col panel
            rc = rcp.tile([P, RPC // 2, 2, 2, MW], f8)
            S = small.tile([P, RPC], f32)  # per-(g,c) row sums
            rdiag = small.tile([P, RPC], f16)  # diag per group (masked later)
            csps = pscst.tile([P, MW], f32)  # col-sum accumulator (PE pair-sums)
            ccbuf = small.tile([P, 2 * MW + 1], f32)  # cc payload
            gbuf = small.tile([P, 2 * MW + 1], f32)  # global result
            cc_in = dram.tile([P, 2 * MW + 1], f32)
            cc_out = dram.tile([P, 2 * MW + 1], f32)

            HB = CHUNK // 2  # row pairs per chunk
            rcflat = rc.rearrange("p b t r m -> p (b t r m)")
            for i in range(NCHUNK):
                r0 = i * CHUNK
                b0 = i * HB
                nc.sync.dma_start(
                    rc[:, b0 : b0 + HB, :, :, :], rc_d[:, b0 : b0 + HB, :, :, :]
                )
                # diag: row r=2b'+r2 at flat offset 1026*b' + 257*r2 + r0
                base = b0 * 4 * MW + r0
                for r2i in range(2):
                    nc.scalar.activation(
                        rdiag[:, r0 + r2i : r0 + CHUNK : 2],
                        rcflat[
                            :,
                            base + 257 * r2i : base + 257 * r2i + 7 * 1026 + 1 : 1026,
                        ],
                        COPY,
                    )
                with nc.allow_low_precision(reason="fp16 partial sums"):
                    # row sums: halving tree over m_w (reads fp8 panel), reduce
                    r1 = treep.tile([P, HB, 2, MW // 2], f16, tag="r1")
                    nc.vector.tensor_tensor(
                        r1[:],
                        rc[:, b0 : b0 + HB, 0, :, 0 : MW // 2],
                        rc[:, b0 : b0 + HB, 0, :, MW // 2 :],
                        op=add,
                    )
                    r2 = treep.tile([P, HB, 2, MW // 4], f16, tag="r2")
                    nc.vector.tensor_tensor(
                        r2[:], r1[:, :, :, 0 : MW // 4], r1[:, :, :, MW // 4 :], op=add
                    )
                    r3 = treep.tile([P, HB, 2, MW // 8], f16, tag="r3")
                    nc.vector.tensor_tensor(
                        r3[:], r2[:, :, :, 0 : MW // 8], r2[:, :, :, MW // 8 :], op=add
                    )
                nc.vector.tensor_reduce(
                    S[:, r0 : r0 + CHUNK].rearrange("p (b r) -> p b r", r=2),
                    r3[:],
                    axis=mybir.AxisListType.X,
                    op=add,
                )
                # col sums on PE: DoubleRow pair-sum copy-matmul accumulates
                # csps[(g,c), m] += rc[., b, 0, 0, m] + rc[., b, 0, 1, m]
                for b in range(b0, b0 + HB):
                    nc.tensor.matmul(
                        csps[:],
                        id8[:],
                        rc[:, b, 0, :, :],
                        start=(b == 0),
                        stop=(b == RPC // 2 - 1),
                        perf_mode=mybir.MatmulPerfMode.DoubleRow,
                        skip_group_check=True,
                    )

            csacc = small.tile([P, MW], f32)
            nc.scalar.activation(csacc[:], csps[:], COPY)

            # ---- pre-folded B table + cc payload, single f32 AllReduce ----
            bps = psstat.tile([P, MW], f32, tag="apck")
            nc.tensor.matmul(bps[:], wb_cs[:], csacc[:], start=True, stop=False)
            nc.tensor.matmul(bps[:], wb_dv[:], rdiag[:], start=False, stop=False)
            nc.tensor.matmul(bps[:], wb_rs[:], S[:], start=False, stop=True)
            nc.scalar.activation(ccbuf[:, 0:MW], bps[:], COPY)
            nc.vector.tensor_copy(ccbuf[:, MW : 2 * MW], csacc[:])
            dcol = treep.tile([P, 1], f32, tag="dcol")
            nc.vector.tensor_reduce(
                dcol[:], rdiag[:], axis=mybir.AxisListType.X, op=add
            )
            nc.vector.tensor_scalar_mul(
                ccbuf[:, 2 * MW : 2 * MW + 1], dcol[:], smask[:]
            )
            nc.gpsimd.dma_start(cc_in[:], ccbuf[:])
            nc.gpsimd.collective_compute(
                "AllReduce",
                add,
                replica_groups=[list(range(NCORES))],
                ins=[cc_in.opt()],
                outs=[cc_out.opt()],
            )
            nc.gpsimd.dma_start(gbuf[:], cc_out[:])

            # ---- post-collective: stacked stat tile, A_packed / Dmask ----
            stats3 = small.tile([96, MW], f32)
            dvp = psstat.tile([P, RPC], f32, tag="apck")
            nc.tensor.matmul(dvp[0:C, 0:MW], gk16[:], rdiag[:], start=True, stop=True)
            nc.scalar.activation(stats3[0:C, :], dvp[0:C, 0:MW], COPY)
            csp = psstat.tile([P, RPC], f32, tag="apck")
            nc.tensor.matmul(
                csp[0:C, 0:MW], gk32[:], gbuf[:, MW : 2 * MW], start=True, stop=True
            )
            nc.scalar.activation(stats3[32:48, :], csp[0:C, 0:MW], COPY)
            rsp = psstat.tile([P, RPC], f32, tag="apck")
            nc.tensor.matmul(rsp[0:C, 0:MW], g_all[:], S[:], start=True, stop=True)
            nc.scalar.activation(stats3[64:80, :], rsp[0:C, 0:MW], COPY)
            # consts rows: 0-15 dsum | 32-47 tsum
            consts = small.tile([48, 1], f32)
            dsp = psstat.tile([P, RPC], f32, tag="apck")
            nc.tensor.matmul(
                dsp[0:C, 0:1], g_all[:], gbuf[:, 2 * MW : 2 * MW + 1],
                start=True, stop=True,
            )
            nc.scalar.activation(consts[0:C, :], dsp[0:C, 0:1], COPY)
            cst2 = small.tile([P, 1], f32)
            nc.vector.tensor_reduce(
                cst2[:], gbuf[:, MW : 2 * MW], axis=mybir.AxisListType.X, op=add
            )
            tsp = psstat.tile([P, RPC], f32, tag="apck")
            nc.tensor.matmul(tsp[0:C, 0:1], g_all[:], cst2[:], start=True, stop=True)
            nc.scalar.activation(consts[32:48, :], tsp[0:C, 0:1], COPY)

            ca = small.tile([P, 1], f32)
            cap = psstat.tile([P, RPC], f32, tag="apck")
            nc.tensor.matmul(cap[:, 0:1], wca2[:], consts[:], start=True, stop=True)
            nc.scalar.activation(ca[:], cap[:, 0:1], IDENT, bias=bsum[:])
            cd = small.tile([P, 1], f32)
            cdp = psstat.tile([P, RPC], f32, tag="apck")
            nc.tensor.matmul(cdp[:, 0:1], wcd2[:], consts[:], start=True, stop=True)
            nc.scalar.activation(cd[:], cdp[:, 0:1], COPY)

            A_packed = small.tile([P, RPC], f32)
            aps = psstat.tile([P, RPC], f32, tag="apck")
            nc.tensor.matmul(aps[:], wa2[:], stats3[:], start=True, stop=True)
            nc.scalar.activation(A_packed[:], aps[:], IDENT, bias=ca[:])
            # A expanded 32x along an inner dim so the vector adds stay packed
            A32 = small.tile([P, RPC, 32], f16)
            apc = A_packed.rearrange("p (n x) -> p n x", x=1)
            with nc.allow_low_precision(reason="fp16 A table"):
                nc.vector.tensor_copy(A32[:], apc.broadcast_to([P, RPC, 32]))
            Dmask = small.tile([P, RPC], f32)
            dps = psstat.tile([P, RPC], f32, tag="apck")
            nc.tensor.matmul(dps[:], wd2[:], stats3[:], start=True, stop=True)
            nc.scalar.activation(Dmask[:], dps[:], IDENT, bias=cd[:])

            # ---- fp8 transpose of the B table for the one-hot PE add ----
            bt8 = small.tile([P, 2, P], f8)
            for mb in range(2):
                btp = psstat.tile([P, MW], f32, tag="apck")
                nc.tensor.matmul(
                    btp[:, 0:P],
                    gbuf[:, mb * P : (mb + 1) * P],
                    idf[:],
                    is_transpose=True,
                    start=True,
                    stop=True,
                    skip_group_check=True,
                )
                with nc.allow_low_precision(reason="fp8 B table"):
                    nc.scalar.activation(bt8[:, mb, :], btp[:, 0:P], COPY)

            # ---- main loop: fused DoubleRow matmuls, split eviction ----
            sthold = [None, None]
            for q in range(NQ):
                r0 = q * QR
                pt = psmain.tile([P, QR, MW], f32)
                for s in range(2):
                    nc.tensor.matmul(
                        pt[:, 2 * s : 2 * s + 2, :],
                        wst[:],
                        rc[:, 2 * q + s, :, :, :],
                        start=True,
                        stop=False,
                        perf_mode=mybir.MatmulPerfMode.DoubleRow,
                    )
                for s in range(2):
                    nc.tensor.matmul(
                        pt[:, 2 * s : 2 * s + 2, :],
                        bt8[:],
                        ohb[:],
                        start=False,
                        stop=True,
                        perf_mode=mybir.MatmulPerfMode.DoubleRow,
                    )
                if q % 2 == 0:
                    st8 = stagep.tile([P, 2 * QR, MW], f16)
                    sthold[0] = st8
                else:
                    st8 = sthold[0]
                st = st8[:, (q % 2) * QR : (q % 2) * QR + QR, :]
                st4 = st.rearrange("p n (u x) -> p n u x", x=32)
                pt4 = pt.rearrange("p n (u x) -> p n u x", x=32)
                k = 3 if q % 2 == 0 else 2  # 5/8 scalar, 3/8 vector
                with nc.allow_low_precision(reason="fp16 output"):
                    nc.scalar.activation(st[:, 0:k, :], pt[:, 0:k, :], COPY)
                    nc.vector.tensor_tensor(
                        st4[:, k:QR, :, :],
                        pt4[:, k:QR, :, :],
                        A32[:, r0 + k : r0 + QR, :]
                        .rearrange("p n (u x) -> p n u x", u=1)
                        .broadcast_to([P, QR - k, MW // 32, 32]),
                        op=add,
                    )
                    nc.vector.tensor_tensor(
                        st4[:, 0:k, :, :],
                        st4[:, 0:k, :, :],
                        A32[:, r0 : r0 + k, :]
                        .rearrange("p n (u x) -> p n u x", u=1)
                        .broadcast_to([P, k, MW // 32, 32]),
                        op=add,
                    )
                    # diagonal fix-up
                    stflat = st.rearrange("p n m -> p (n m)")
                    nc.vector.tensor_tensor(
                        stflat[:, r0 : r0 + (QR - 1) * (MW + 1) + 1 : MW + 1],
                        stflat[:, r0 : r0 + (QR - 1) * (MW + 1) + 1 : MW + 1],
                        Dmask[:, r0 : r0 + QR],
                        op=add,
                    )
                if q % 2 == 1:
                    eng = nc.sync if (q // 2) % 2 == 0 else nc.gpsimd
                    eng.dma_start(y_d[:, r0 - QR : r0 + QR, :], st8[:])

    nc.compile()
    return nc


def _host_prep(X, weights, bias):
    """Pack panels + fold weights into per-core input maps."""
    import ml_dtypes

    W = weights.astype(np.float32)
    iN = np.float32(1.0 / N)
    iN2 = np.float32(1.0 / (N * N))
    bias_sum = np.float32(bias.astype(np.float64).sum())

    Xr = np.ascontiguousarray(X[0])  # [C, N, N] fp32
    Rp = (
        Xr.reshape(C, NCORES, RPC, G, MW)
        .transpose(1, 3, 0, 2, 4)
        .reshape(NCORES, P, RPC, MW)
        .astype(ml_dtypes.float8_e4m3)
    )
    XT = np.ascontiguousarray(Xr.transpose(0, 2, 1))
    Cp = (
        XT.reshape(C, NCORES, RPC, G, MW)
        .transpose(1, 3, 0, 2, 4)
        .reshape(NCORES, P, RPC, MW)
        .astype(ml_dtypes.float8_e4m3)
    )
    # interleave: rc8[k, p, b, t, r2, m], row = 2b + r2; t=0 rows, t=1 cols
    rc8 = np.ascontiguousarray(
        np.stack(
            [
                Rp.reshape(NCORES, P, RPC // 2, 2, MW),
                Cp.reshape(NCORES, P, RPC // 2, 2, MW),
            ],
            axis=3,
        )
    )

    def blockdiag(w, dtype):
        out = np.zeros((P, P), dtype=dtype)
        for g in range(G):
            out[g * C : (g + 1) * C, g * C : (g + 1) * C] = w
        return out

    wst = np.stack(
        [
            blockdiag(W[8], np.float32),
            blockdiag(W[6], np.float32),
        ],
        axis=1,
    ).astype(ml_dtypes.float8_e4m3)  # [128, 2, 128]

    # one-hot B pattern: ohb[k, t, r2, m] = (m == 128*t + k)
    ohb = np.zeros((P, 2, 2, MW), np.float32)
    for t in range(2):
        for k in range(P):
            ohb[k, t, :, t * P + k] = 1.0
    ohb = ohb.astype(ml_dtypes.float8_e4m3)

    id8 = np.stack([np.eye(P, dtype=np.float32)] * 2, axis=1).astype(
        ml_dtypes.float8_e4m3
    )  # [128, 2, 128] stacked identities for the pair-sum matmul
    idf = np.eye(P, dtype=np.float32)
    wb_cs = blockdiag(W[10] * iN, np.float32)
    g_all = np.tile(np.eye(C, dtype=np.float32), (G, 1))  # [128, 16]

    def rep(w):  # [16,16] -> [16, 128]
        return np.tile(w.astype(np.float32), (1, G))

    in_maps = []
    for k in range(NCORES):
        rowmask = np.repeat((np.arange(G) == k).astype(np.float32), C)  # [128]
        gk = g_all * rowmask[:, None]
        wb_dv = blockdiag(W[9], np.float32) * rowmask[:, None]
        wb_rs = np.tile(W[13] * iN, (G, G)) * rowmask[None, :]
        wad = np.zeros((2, 96, P), np.float32)
        wad[0, 0:C] = rep(W[5])
        wad[0, 32:48] = rep(W[7] * iN)
        wad[0, 64:80] = rep(W[12] * iN)
        wad[1, 0:C] = rep(W[0]) * rowmask[None, :]
        wad[1, 32:48] = rep(W[1] * iN) * rowmask[None, :]
        wad[1, 64:80] = rep(W[3] * iN) * rowmask[None, :]
        wcc = np.zeros((2, 48, P), np.float32)
        wcc[0, 0:C] = rep(W[11] * iN)
        wcc[0, 32:48] = rep(W[14] * iN2)
        wcc[1, 0:C] = rep(W[2] * iN) * rowmask[None, :]
        wcc[1, 32:48] = rep(W[4] * iN2) * rowmask[None, :]
        in_maps.append(
            {
                "rc8": rc8[k],
                "wst": wst,
                "ohb": ohb,
                "id8": id8,
                "idf": idf,
                "wb_cs": wb_cs,
                "wb_dv": wb_dv.astype(np.float16),
                "wb_rs": wb_rs,
                "gk16": gk.astype(np.float16),
                "gk32": gk,
                "g_all": g_all,
                "wad": wad,
                "wcc": wcc,
                "smask": rowmask[:, None].copy(),
                "bsum": np.full((P, 1), bias_sum, np.float32),
            }
        )
    return in_maps


def kernel(X, weights, bias):
    if "nc" not in _CACHED:
        _CACHED["nc"] = _build_program()
    nc = _CACHED["nc"]

    trace = bool(os.environ.get("BASS_TRACE"))
    if trace:
        _install_trace_hook()

    in_maps = _host_prep(np.asarray(X), np.asarray(weights), np.asarray(bias))
    res = bass_utils.run_bass_kernel_spmd(
        nc, in_maps, core_ids=list(range(NCORES)), trace=trace
    )
    LAST_RUN_INFO.clear()
    LAST_RUN_INFO.update(
        exec_time_ns=res.exec_time_ns,
        mean_exec_time_ns=res.mean_exec_time_ns,
        trace=res.instructions_and_trace[1] if res.instructions_and_trace else None,
    )

    Yp = np.stack([np.asarray(res.results[k]["y"]) for k in range(NCORES)])
    Y = (
        Yp.astype(np.float32)
        .reshape(NCORES, G, C, RPC, MW)
        .transpose(2, 0, 3, 1, 4)
        .reshape(1, C, N, N)
    )
    return Y



# revision 9
# speedup vs baseline: 1.3674x; 1.3674x over previous
"""Equivariant layer block (order-2, 15-basis) on 8 Trainium2 NeuronCores.

Decomposition (indices: c in-channel, o out-channel, n/m spatial, N=2048):
  Y[o,n,m] = sum_c X[c,n,m] W8[c,o] + X[c,m,n] W6[c,o]
           + A[o,n] + B[o,m] + D[o,n] delta[n,m]
with (raw sums; /N factors folded into host-side weights; i = ref basis index)
  A[o,n] = dv.W5 + csum.W7/N + rsum.W12/N + dsum.W11/N + tsum.W14/N^2
  B[o,m] = dv.W9 + csum.W10/N + rsum.W13/N + sum(bias)
  D[o,n] = dv.W0 + csum.W1/N + rsum.W3/N + dsum.W2/N + tsum.W4/N^2

v4 design. Core k owns output rows I_k=[256k,256k+256). Both spatial panels
stream through a 4-chunk fp8 SBUF ring rc[(g,c), b, t, r2, m] (t=0 rows,
t=1 cols); per chunk the PE does the fused DoubleRow main matmul (identity +
transpose terms via the two k-tiles) plus identity pair-sum matmuls that
accumulate partial column sums (from the row panel) AND partial row sums
(from the column panel) in one persistent PSUM bank. The main term is
evicted table-free into a full f16 staging tile Y0 (Act/DVE alternating
plain copies), so the whole load phase is DMA-bound and the PE never
switches stationary weights mid-phase.

One f16 AllReduce of [128, 770] (pre-folded B table | csum partials | rsum
partials | masked diag column) fires as soon as the last chunk's pair-sums
land. Post-collective, a short matmul chain builds the A/B/D tables, then
pass 2 walks Y0 in 8-row blocks: two DVE 4x-mode broadcast adds (+A along
m, +B along r), a tiny Pool diag fix-up, and pipelined f16 stores on the
sync/gpsimd queues. sum(bias) is folded into B on device.
"""

import os
import numpy as np

import concourse.bacc as bacc
import concourse.tile as tile
import concourse.mybir as mybir
from concourse import bass_utils

N = 2048
C = 16
NCORES = 8
RPC = N // NCORES  # 256 rows per core
G = 8  # m-groups
MW = N // G  # 256
P = 128
CHUNK = 16  # rows per DMA chunk
NCHUNK = RPC // CHUNK  # 16
HB = CHUNK // 2  # row-pairs per chunk (8)
QR = 4  # rows per main-loop quarter-chunk
NQ = RPC // QR  # 64
BR = 8  # rows per pass-2 block
NB = RPC // BR  # 32
AXU = 8  # inner expansion of the A table (packed last dim for DVE 4x)
CCW = 2 * 256 + 256 + 2  # AllReduce payload cols: B_pre|csum|rsum|dcol|pad
f16 = mybir.dt.float16
f32 = mybir.dt.float32
f8 = mybir.dt.float8e4

LAST_RUN_INFO = {}
_CACHED = {}


def _install_trace_hook():
    """Best-effort NTFF hook injection (used only when BASS_TRACE is set)."""
    try:
        import sys, types

        if "antenv.axon_hooks" in sys.modules:
            return
        mod = types.ModuleType("antenv.axon_hooks")
        state = {}
        mod.set_axon_ntff_profile_hook = lambda h: state.update(h=h)
        mod.get_axon_ntff_profile_hook = lambda: state.get("h")
        sys.modules["antenv.axon_hooks"] = mod
        import antenv

        antenv.axon_hooks = mod
        from trn_agent_boot.trn_boot import _ntff_profile_via_ctypes

        mod.set_axon_ntff_profile_hook(
            _ntff_profile_via_ctypes("/opt/axon/libaxon_pjrt.so")
        )
    except Exception:
        pass


def _build_program():
    nc = bacc.Bacc("TRN2", target_bir_lowering=False, debug=False, num_devices=NCORES)

    # interleaved panel: rc_d[p, b, t, r2, m], row = 2b+r2, t=0 rows / t=1 cols
    rc_d = nc.dram_tensor("rc8", [P, RPC // 2, 2, 2, MW], f8, kind="ExternalInput").ap()
    wst_d = nc.dram_tensor("wst", [P, 2, P], f8, kind="ExternalInput").ap()
    id8_d = nc.dram_tensor("id8", [P, 2, P], f8, kind="ExternalInput").ap()
    wbcs_d = nc.dram_tensor("wb_cs", [P, P], f32, kind="ExternalInput").ap()
    wbdv_d = nc.dram_tensor("wb_dv", [P, P], f16, kind="ExternalInput").ap()
    wbrs_d = nc.dram_tensor("wb_rs", [P, P], f32, kind="ExternalInput").ap()
    gk16_d = nc.dram_tensor("gk16", [P, C], f16, kind="ExternalInput").ap()
    gall_d = nc.dram_tensor("g_all", [P, C], f16, kind="ExternalInput").ap()
    wad_d = nc.dram_tensor("wad", [2, 96, P], f32, kind="ExternalInput").ap()
    wcc_d = nc.dram_tensor("wcc", [2, 48, P], f32, kind="ExternalInput").ap()
    smask_d = nc.dram_tensor("smask", [P, 1], f32, kind="ExternalInput").ap()
    bsum_d = nc.dram_tensor("bsum", [P, 1], f32, kind="ExternalInput").ap()

    y_d = nc.dram_tensor("y", [P, RPC, MW], f16, kind="ExternalOutput").ap()

    add = mybir.AluOpType.add
    COPY = mybir.ActivationFunctionType.Copy
    IDENT = mybir.ActivationFunctionType.Identity

    with tile.TileContext(nc) as tc:
        with (
            tc.tile_pool(name="small", bufs=1) as small,
            tc.tile_pool(name="rcring", bufs=4) as rcring,
            tc.tile_pool(name="y0p", bufs=1) as y0p,
            tc.tile_pool(name="stagep", bufs=4) as stagep,
            tc.tile_pool(name="pscr", bufs=1, space="PSUM") as pscr,
            tc.tile_pool(name="psstat", bufs=1, space="PSUM") as psstat,
            tc.tile_pool(name="psmain", bufs=3, space="PSUM") as psmain,
            tc.tile_pool(name="dram", bufs=1, space="DRAM") as dram,
        ):
            # ---- constant / weight loads ----
            wst = small.tile([P, 2, P], f8)
            id8 = small.tile([P, 2, P], f8)
            wb_cs = small.tile([P, P], f32)
            wb_dv = small.tile([P, P], f16)
            wb_rs = small.tile([P, P], f32)
            gk16 = small.tile([P, C], f16)
            g_all = small.tile([P, C], f16)
            smask = small.tile([P, 1], f32)
            bsum = small.tile([P, 1], f32)
            for t, d in [
                (wst, wst_d),
                (id8, id8_d),
                (wb_cs, wbcs_d),
                (wb_dv, wbdv_d),
                (wb_rs, wbrs_d),
                (gk16, gk16_d),
                (g_all, gall_d),
                (smask, smask_d),
                (bsum, bsum_d),
            ]:
                nc.sync.dma_start(t[:], d[:])
            wa2 = small.tile([96, P], f32)
            wd2 = small.tile([96, P], f32)
            wca2 = small.tile([48, P], f32)
            wcd2 = small.tile([48, P], f32)
            nc.sync.dma_start(wa2[:], wad_d[0])
            nc.sync.dma_start(wd2[:], wad_d[1])
            nc.sync.dma_start(wca2[:], wcc_d[0])
            nc.sync.dma_start(wcd2[:], wcc_d[1])

            Y0 = y0p.tile([P, RPC, MW], f16)  # staged main term (pre-tables)
            rdiag = small.tile([P, RPC], f16)  # diag per group (g=k rows valid)
            # csum/rsum pair-sum accumulators share one PSUM bank: [cs | rs]
            csrs = pscr.tile([P, 2 * MW], f32)
            ccbuf = small.tile([P, CCW], f16)
            gbuf = small.tile([P, CCW], f16)
            cc_in = dram.tile([P, CCW], f16)
            cc_out = dram.tile([P, CCW], f16)

            # ---- load phase: stream chunks, stats + main matmuls + evict ----
            for i in range(NCHUNK):
                r0 = i * CHUNK
                b0 = i * HB
                rct = rcring.tile([P, HB, 2, 2, MW], f8, tag="rc")
                nc.sync.dma_start(rct[:], rc_d[:, b0 : b0 + HB, :, :, :])
                # diag extract: row r=2b'+r2 at flat offset 1026*b' + 257*r2
                rcflat = rct.rearrange("p b t r m -> p (b t r m)")
                for r2i in range(2):
                    nc.scalar.activation(
                        rdiag[:, r0 + r2i : r0 + CHUNK : 2],
                        rcflat[:, 257 * r2i + r0 : 257 * r2i + r0 + 7 * 1026 + 1 : 1026],
                        COPY,
                    )
                # pair-sum stats on PE: csrs[:,0:MW] += R rows, csrs[:,MW:] += C rows
                for b in range(HB):
                    gb = b0 + b
                    nc.tensor.matmul(
                        csrs[:, 0:MW],
                        id8[:],
                        rct[:, b, 0, :, :],
                        start=(gb == 0),
                        stop=(gb == RPC // 2 - 1),
                        perf_mode=mybir.MatmulPerfMode.DoubleRow,
                        skip_group_check=True,
                    )
                    nc.tensor.matmul(
                        csrs[:, MW : 2 * MW],
                        id8[:],
                        rct[:, b, 1, :, :],
                        start=(gb == 0),
                        stop=(gb == RPC // 2 - 1),
                        perf_mode=mybir.MatmulPerfMode.DoubleRow,
                        skip_group_check=True,
                    )
                # main term: 4 q per chunk, 2 row-pairs each, evict to Y0
                for s in range(QR):
                    q = i * QR + s
                    qr0 = q * QR
                    pt = psmain.tile([P, QR, MW], f32)
                    for j in range(2):
                        nc.tensor.matmul(
                            pt[:, 2 * j : 2 * j + 2, :],
                            wst[:],
                            rct[:, 2 * s + j, :, :, :],
                            start=True,
                            stop=True,
                            perf_mode=mybir.MatmulPerfMode.DoubleRow,
                        )
                    with nc.allow_low_precision(reason="f16 staging"):
                        if q % 2 == 0:
                            nc.scalar.activation(
                                Y0[:, qr0 : qr0 + QR, :], pt[:], COPY
                            )
                        else:
                            nc.vector.tensor_copy(Y0[:, qr0 : qr0 + QR, :], pt[:])

            # ---- pre-collective fold: B_pre + payload assembly ----
            csr32 = small.tile([P, 2 * MW], f32)
            nc.scalar.activation(csr32[:], csrs[:], COPY)
            bps = psstat.tile([P, MW], f32, tag="apck")
            nc.tensor.matmul(bps[:], wb_cs[:], csr32[:, 0:MW], start=True, stop=False)
            nc.tensor.matmul(bps[:], wb_rs[:], csr32[:, MW:], start=False, stop=False)
            nc.tensor.matmul(bps[:], wb_dv[:], rdiag[:], start=False, stop=True)
            with nc.allow_low_precision(reason="f16 collective payload"):
                nc.scalar.activation(ccbuf[:, 0:MW], bps[:], COPY)
                nc.vector.tensor_copy(ccbuf[:, MW : 3 * MW], csr32[:])
                dcol = small.tile([P, 1], f32)
                nc.vector.tensor_reduce(
                    dcol[:], rdiag[:], axis=mybir.AxisListType.X, op=add
                )
                nc.vector.tensor_scalar_mul(
                    ccbuf[:, 3 * MW : 3 * MW + 1], dcol[:], smask[:]
                )
                nc.vector.memset(ccbuf[:, 3 * MW + 1 : CCW], 0.0)
            nc.gpsimd.dma_start(cc_in[:], ccbuf[:])
            nc.gpsimd.collective_compute(
                "AllReduce",
                add,
                replica_groups=[list(range(NCORES))],
                ins=[cc_in.opt()],
                outs=[cc_out.opt()],
            )
            nc.gpsimd.dma_start(gbuf[:], cc_out[:])

            # ---- post-collective: A/B/D tables ----
            g_cs = gbuf[:, MW : 2 * MW]
            g_rs = gbuf[:, 2 * MW : 3 * MW]
            B16 = small.tile([P, MW], f16)
            with nc.allow_low_precision(reason="f16 B table"):
                nc.scalar.activation(B16[:], gbuf[:, 0:MW], IDENT, bias=bsum[:])
            stats3 = small.tile([96, MW], f32)
            dvp = psstat.tile([P, MW], f32, tag="apck")
            nc.tensor.matmul(dvp[0:C, :], gk16[:], rdiag[:], start=True, stop=True)
            nc.scalar.activation(stats3[0:C, :], dvp[0:C, :], COPY)
            csp = psstat.tile([P, MW], f32, tag="apck")
            nc.tensor.matmul(csp[0:C, :], gk16[:], g_cs, start=True, stop=True)
            nc.scalar.activation(stats3[32:48, :], csp[0:C, :], COPY)
            rsp = psstat.tile([P, MW], f32, tag="apck")
            nc.tensor.matmul(rsp[0:C, :], gk16[:], g_rs, start=True, stop=True)
            nc.scalar.activation(stats3[64:80, :], rsp[0:C, :], COPY)
            # consts rows: 0-15 dsum | 32-47 tsum
            consts = small.tile([48, 1], f32)
            dsp = psstat.tile([P, MW], f32, tag="apck")
            nc.tensor.matmul(
                dsp[0:C, 0:1], g_all[:], gbuf[:, 3 * MW : 3 * MW + 1],
                start=True, stop=True,
            )
            nc.scalar.activation(consts[0:C, :], dsp[0:C, 0:1], COPY)
            cst2 = small.tile([P, 1], f16)
            with nc.allow_low_precision(reason="f16 total-sum scalar"):
                nc.vector.tensor_reduce(
                    cst2[:], g_cs, axis=mybir.AxisListType.X, op=add
                )
            tsp = psstat.tile([P, MW], f32, tag="apck")
            nc.tensor.matmul(tsp[0:C, 0:1], g_all[:], cst2[:], start=True, stop=True)
            nc.scalar.activation(consts[32:48, :], tsp[0:C, 0:1], COPY)

            ca = small.tile([P, 1], f32)
            cap = psstat.tile([P, MW], f32, tag="apck")
            nc.tensor.matmul(cap[:, 0:1], wca2[:], consts[:], start=True, stop=True)
            nc.scalar.activation(ca[:], cap[:, 0:1], COPY)
            cd = small.tile([P, 1], f32)
            cdp = psstat.tile([P, MW], f32, tag="apck")
            nc.tensor.matmul(cdp[:, 0:1], wcd2[:], consts[:], start=True, stop=True)
            nc.scalar.activation(cd[:], cdp[:, 0:1], COPY)

            A16 = small.tile([P, RPC], f16)
            aps = psstat.tile([P, MW], f32, tag="apck")
            nc.tensor.matmul(aps[:], wa2[:], stats3[:], start=True, stop=True)
            with nc.allow_low_precision(reason="f16 A table"):
                nc.scalar.activation(A16[:], aps[:], IDENT, bias=ca[:])
            # A expanded AXU-wide so pass-2 keeps a packed last dim (DVE 4x)
            A8e = small.tile([P, RPC, AXU], f16)
            with nc.allow_low_precision(reason="f16 A table"):
                nc.vector.tensor_copy(
                    A8e[:],
                    A16.rearrange("p (n x) -> p n x", x=1).broadcast_to(
                        [P, RPC, AXU]
                    ),
                )
            Dm16 = small.tile([P, RPC], f16)
            dps = psstat.tile([P, MW], f32, tag="apck")
            nc.tensor.matmul(dps[:], wd2[:], stats3[:], start=True, stop=True)
            with nc.allow_low_precision(reason="f16 D table"):
                nc.scalar.activation(Dm16[:], dps[:], IDENT, bias=cd[:])

            # ---- pass 2: +A (bcast m), +B (bcast r), diag fix, store ----
            for qq in range(NB):
                r0 = qq * BR
                y0b = Y0[:, r0 : r0 + BR, :]
                y0b4 = y0b.rearrange("p n (u x) -> p n u x", x=AXU)
                with nc.allow_low_precision(reason="f16 output"):
                    nc.vector.tensor_tensor(
                        y0b4[:],
                        y0b4[:],
                        A8e[:, r0 : r0 + BR, :]
                        .rearrange("p n (u x) -> p n u x", u=1)
                        .broadcast_to([P, BR, MW // AXU, AXU]),
                        op=add,
                    )
                    st = stagep.tile([P, BR, MW], f16, tag="st")
                    nc.vector.tensor_tensor(
                        st[:],
                        y0b[:],
                        B16.rearrange("p (n m) -> p n m", n=1).broadcast_to(
                            [P, BR, MW]
                        ),
                        op=add,
                    )
                    # diagonal fix-up (valid only on g=k partitions; Dm16
                    # is masked to zero elsewhere)
                    stflat = st.rearrange("p n m -> p (n m)")
                    nc.gpsimd.tensor_tensor(
                        stflat[:, r0 : r0 + (BR - 1) * (MW + 1) + 1 : MW + 1],
                        stflat[:, r0 : r0 + (BR - 1) * (MW + 1) + 1 : MW + 1],
                        Dm16[:, r0 : r0 + BR],
                        op=add,
                    )
                eng = nc.sync if qq % 2 == 0 else nc.gpsimd
                eng.dma_start(y_d[:, r0 : r0 + BR, :], st[:])

    nc.compile()
    return nc


def _host_prep(X, weights, bias):
    """Pack panels + fold weights into per-core input maps."""
    import ml_dtypes

    W = weights.astype(np.float32)
    iN = np.float32(1.0 / N)
    iN2 = np.float32(1.0 / (N * N))
    bias_sum = np.float32(bias.astype(np.float64).sum())

    Xr = np.ascontiguousarray(X[0])  # [C, N, N] fp32
    Rp = (
        Xr.reshape(C, NCORES, RPC, G, MW)
        .transpose(1, 3, 0, 2, 4)
        .reshape(NCORES, P, RPC, MW)
        .astype(ml_dtypes.float8_e4m3)
    )
    XT = np.ascontiguousarray(Xr.transpose(0, 2, 1))
    Cp = (
        XT.reshape(C, NCORES, RPC, G, MW)
        .transpose(1, 3, 0, 2, 4)
        .reshape(NCORES, P, RPC, MW)
        .astype(ml_dtypes.float8_e4m3)
    )
    # interleave: rc8[k, p, b, t, r2, m], row = 2b + r2; t=0 rows, t=1 cols
    rc8 = np.ascontiguousarray(
        np.stack(
            [
                Rp.reshape(NCORES, P, RPC // 2, 2, MW),
                Cp.reshape(NCORES, P, RPC // 2, 2, MW),
            ],
            axis=3,
        )
    )

    def blockdiag(w, dtype):
        out = np.zeros((P, P), dtype=dtype)
        for g in range(G):
            out[g * C : (g + 1) * C, g * C : (g + 1) * C] = w
        return out

    wst = np.stack(
        [
            blockdiag(W[8], np.float32),
            blockdiag(W[6], np.float32),
        ],
        axis=1,
    ).astype(ml_dtypes.float8_e4m3)  # [128, 2, 128]

    id8 = np.stack([np.eye(P, dtype=np.float32)] * 2, axis=1).astype(
        ml_dtypes.float8_e4m3
    )  # [128, 2, 128] stacked identities for the pair-sum matmuls
    wb_cs = blockdiag(W[10] * iN, np.float32)
    wb_rs = blockdiag(W[13] * iN, np.float32)
    g_all = np.tile(np.eye(C, dtype=np.float32), (G, 1))  # [128, 16]

    def rep(w):  # [16,16] -> [16, 128]
        return np.tile(w.astype(np.float32), (1, G))

    in_maps = []
    for k in range(NCORES):
        rowmask = np.repeat((np.arange(G) == k).astype(np.float32), C)  # [128]
        gk = g_all * rowmask[:, None]
        wb_dv = blockdiag(W[9], np.float32) * rowmask[:, None]
        wad = np.zeros((2, 96, P), np.float32)
        wad[0, 0:C] = rep(W[5])
        wad[0, 32:48] = rep(W[7] * iN)
        wad[0, 64:80] = rep(W[12] * iN)
        wad[1, 0:C] = rep(W[0]) * rowmask[None, :]
        wad[1, 32:48] = rep(W[1] * iN) * rowmask[None, :]
        wad[1, 64:80] = rep(W[3] * iN) * rowmask[None, :]
        wcc = np.zeros((2, 48, P), np.float32)
        wcc[0, 0:C] = rep(W[11] * iN)
        wcc[0, 32:48] = rep(W[14] * iN2)
        wcc[1, 0:C] = rep(W[2] * iN) * rowmask[None, :]
        wcc[1, 32:48] = rep(W[4] * iN2) * rowmask[None, :]
        in_maps.append(
            {
                "rc8": rc8[k],
                "wst": wst,
                "id8": id8,
                "wb_cs": wb_cs,
                "wb_dv": wb_dv.astype(np.float16),
                "wb_rs": wb_rs,
                "gk16": gk.astype(np.float16),
                "g_all": g_all.astype(np.float16),
                "wad": wad,
                "wcc": wcc,
                "smask": rowmask[:, None].copy(),
                "bsum": np.full((P, 1), bias_sum, np.float32),
            }
        )
    return in_maps


def kernel(X, weights, bias):
    if "nc" not in _CACHED:
        _CACHED["nc"] = _build_program()
    nc = _CACHED["nc"]

    trace = bool(os.environ.get("BASS_TRACE"))
    if trace:
        _install_trace_hook()

    in_maps = _host_prep(np.asarray(X), np.asarray(weights), np.asarray(bias))
    res = bass_utils.run_bass_kernel_spmd(
        nc, in_maps, core_ids=list(range(NCORES)), trace=trace
    )
    LAST_RUN_INFO.clear()
    LAST_RUN_INFO.update(
        exec_time_ns=res.exec_time_ns,
        mean_exec_time_ns=res.mean_exec_time_ns,
        trace=res.instructions_and_trace[1] if res.instructions_and_trace else None,
    )

    Yp = np.stack([np.asarray(res.results[k]["y"]) for k in range(NCORES)])
    Y = (
        Yp.astype(np.float32)
        .reshape(NCORES, G, C, RPC, MW)
        .transpose(2, 0, 3, 1, 4)
        .reshape(1, C, N, N)
    )
    return Y


# revision 26
# speedup vs baseline: 1.3930x; 1.0188x over previous
"""Equivariant layer block (order-2, 15-basis) on 8 Trainium2 NeuronCores.

Decomposition (indices: c in-channel, o out-channel, n/m spatial, N=2048):
  Y[o,n,m] = sum_c X[c,n,m] W8[c,o] + X[c,m,n] W6[c,o]
           + A[o,n] + B[o,m] + D[o,n] delta[n,m]
with (raw sums; /N factors folded into host-side weights; i = ref basis index)
  A[o,n] = dv.W5 + csum.W7/N + rsum.W12/N + dsum.W11/N + tsum.W14/N^2
  B[o,m] = dv.W9 + csum.W10/N + rsum.W13/N + sum(bias)
  D[o,n] = dv.W0 + csum.W1/N + rsum.W3/N + dsum.W2/N + tsum.W4/N^2

v4 design. Core k owns output rows I_k=[256k,256k+256). Both spatial panels
stream through a 4-chunk fp8 SBUF ring rc[(g,c), b, t, r2, m] (t=0 rows,
t=1 cols); per chunk the PE does the fused DoubleRow main matmul (identity +
transpose terms via the two k-tiles) plus identity pair-sum matmuls that
accumulate partial column sums (from the row panel) AND partial row sums
(from the column panel) in one persistent PSUM bank. The main term is
evicted table-free into a full f16 staging tile Y0 (Act/DVE alternating
plain copies), so the whole load phase is DMA-bound and the PE never
switches stationary weights mid-phase.

One f16 AllReduce of [128, 770] (pre-folded B table | csum partials | rsum
partials | masked diag column) fires as soon as the last chunk's pair-sums
land. Post-collective, a short matmul chain builds the A/B/D tables, then
pass 2 walks Y0 in 8-row blocks: two DVE 4x-mode broadcast adds (+A along
m, +B along r), a tiny Pool diag fix-up, and pipelined f16 stores on the
sync/gpsimd queues. sum(bias) is folded into B on device.
"""

import os
import numpy as np

import concourse.bacc as bacc
import concourse.tile as tile
import concourse.mybir as mybir
from concourse import bass_utils

N = 2048
C = 16
NCORES = 8
RPC = N // NCORES  # 256 rows per core
G = 8  # m-groups
MW = N // G  # 256
P = 128
CHUNK = 16  # rows per DMA chunk
NCHUNK = RPC // CHUNK  # 16
HB = CHUNK // 2  # row-pairs per chunk (8)
QR = 4  # rows per main-loop quarter-chunk
NQ = RPC // QR  # 64
BR = 8  # rows per pass-2 block
NB = RPC // BR  # 32
AXU = 8  # inner expansion of the A table (packed last dim for DVE 4x)
CCW = 2 * 256 + 256 + 2  # AllReduce payload cols: B_pre|csum|rsum|dcol|pad
f16 = mybir.dt.float16
f32 = mybir.dt.float32
f8 = mybir.dt.float8e4

LAST_RUN_INFO = {}
_CACHED = {}


def _install_trace_hook():
    """Best-effort NTFF hook injection (used only when BASS_TRACE is set)."""
    try:
        import sys, types

        if "antenv.axon_hooks" in sys.modules:
            return
        mod = types.ModuleType("antenv.axon_hooks")
        state = {}
        mod.set_axon_ntff_profile_hook = lambda h: state.update(h=h)
        mod.get_axon_ntff_profile_hook = lambda: state.get("h")
        sys.modules["antenv.axon_hooks"] = mod
        import antenv

        antenv.axon_hooks = mod
        from trn_agent_boot.trn_boot import _ntff_profile_via_ctypes

        mod.set_axon_ntff_profile_hook(
            _ntff_profile_via_ctypes("/opt/axon/libaxon_pjrt.so")
        )
    except Exception:
        pass


def _build_program():
    nc = bacc.Bacc("TRN2", target_bir_lowering=False, debug=False, num_devices=NCORES)

    # interleaved panel: rc_d[p, b, t, r2, m], row = 2b+r2, t=0 rows / t=1 cols
    rc_d = nc.dram_tensor("rc8", [P, RPC // 2, 2, 2, MW], f8, kind="ExternalInput").ap()
    wst_d = nc.dram_tensor("wst", [P, 2, P], f8, kind="ExternalInput").ap()
    id8_d = nc.dram_tensor("id8", [P, 2, P], f8, kind="ExternalInput").ap()
    ohb_d = nc.dram_tensor("ohb", [P, 2, 2, MW], f8, kind="ExternalInput").ap()
    idt_d = nc.dram_tensor("idt", [P, P], f16, kind="ExternalInput").ap()
    wbcs_d = nc.dram_tensor("wb_cs", [P, P], f32, kind="ExternalInput").ap()
    wbdv_d = nc.dram_tensor("wb_dv", [P, P], f16, kind="ExternalInput").ap()
    wbrs_d = nc.dram_tensor("wb_rs", [P, P], f32, kind="ExternalInput").ap()
    gk16_d = nc.dram_tensor("gk16", [P, C], f16, kind="ExternalInput").ap()
    gall_d = nc.dram_tensor("g_all", [P, C], f16, kind="ExternalInput").ap()
    wad_d = nc.dram_tensor("wad", [2, 96, P], f32, kind="ExternalInput").ap()
    wcc_d = nc.dram_tensor("wcc", [2, 48, P], f32, kind="ExternalInput").ap()
    smask_d = nc.dram_tensor("smask", [P, 1], f32, kind="ExternalInput").ap()
    bsum_d = nc.dram_tensor("bsum", [P, 1], f32, kind="ExternalInput").ap()

    y_d = nc.dram_tensor("y", [P, RPC, MW], f16, kind="ExternalOutput").ap()

    add = mybir.AluOpType.add
    COPY = mybir.ActivationFunctionType.Copy
    IDENT = mybir.ActivationFunctionType.Identity

    with tile.TileContext(nc) as tc:
        with (
            tc.tile_pool(name="small", bufs=1) as small,
            tc.tile_pool(name="rcring", bufs=4) as rcring,
            tc.tile_pool(name="y0p", bufs=1) as y0p,
            tc.tile_pool(name="stagep", bufs=4) as stagep,
            tc.tile_pool(name="pscr", bufs=1, space="PSUM") as pscr,
            tc.tile_pool(name="psstat", bufs=1, space="PSUM") as psstat,
            tc.tile_pool(name="psmain", bufs=3, space="PSUM") as psmain,
            tc.tile_pool(name="dram", bufs=1, space="DRAM") as dram,
        ):
            # ---- constant / weight loads ----
            wst = small.tile([P, 2, P], f8)
            id8 = small.tile([P, 2, P], f8)
            ohb = small.tile([P, 2, 2, MW], f8)
            idt = small.tile([P, P], f16)
            wb_cs = small.tile([P, P], f32)
            wb_dv = small.tile([P, P], f16)
            wb_rs = small.tile([P, P], f32)
            gk16 = small.tile([P, C], f16)
            g_all = small.tile([P, C], f16)
            smask = small.tile([P, 1], f32)
            bsum = small.tile([P, 1], f32)
            for t, d in [
                (wst, wst_d),
                (id8, id8_d),
                (ohb, ohb_d),
                (idt, idt_d),
                (wb_cs, wbcs_d),
                (wb_dv, wbdv_d),
                (wb_rs, wbrs_d),
                (gk16, gk16_d),
                (g_all, gall_d),
                (smask, smask_d),
                (bsum, bsum_d),
            ]:
                nc.sync.dma_start(t[:], d[:])
            wa2 = small.tile([96, P], f32)
            wd2 = small.tile([96, P], f32)
            wca2 = small.tile([48, P], f32)
            wcd2 = small.tile([48, P], f32)
            nc.sync.dma_start(wa2[:], wad_d[0])
            nc.sync.dma_start(wd2[:], wad_d[1])
            nc.sync.dma_start(wca2[:], wcc_d[0])
            nc.sync.dma_start(wcd2[:], wcc_d[1])

            Y0 = y0p.tile([P, RPC, MW], f16)  # staged main term (pre-tables)
            rdiag = small.tile([P, RPC], f16)  # diag per group (g=k rows valid)
            # csum/rsum pair-sum accumulators share one PSUM bank: [cs | rs]
            csrs = pscr.tile([P, 2 * MW], f32)
            ccbuf = small.tile([P, CCW], f16)
            gbuf = small.tile([P, CCW], f16)
            cc_in = dram.tile([P, CCW], f16)
            cc_out = dram.tile([P, CCW], f16)

            # ---- load phase: stream chunks, stats + main matmuls + evict ----
            for i in range(NCHUNK):
                r0 = i * CHUNK
                b0 = i * HB
                rct = rcring.tile([P, HB, 2, 2, MW], f8, tag="rc")
                nc.sync.dma_start(rct[:], rc_d[:, b0 : b0 + HB, :, :, :])
                # diag extract: row r=2b'+r2 at flat offset 1026*b' + 257*r2
                rcflat = rct.rearrange("p b t r m -> p (b t r m)")
                for r2i in range(2):
                    nc.scalar.activation(
                        rdiag[:, r0 + r2i : r0 + CHUNK : 2],
                        rcflat[:, 257 * r2i + r0 : 257 * r2i + r0 + 7 * 1026 + 1 : 1026],
                        COPY,
                    )
                # pair-sum stats on PE: one DoubleRow matmul per row-pair
                # covers BOTH panels (pair over r2; free dims [t, m]):
                # csrs[(g,c), t*MW+m] += rct[.., b, t, 0, m] + rct[.., b, t, 1, m]
                csrs2 = csrs.rearrange("p (t m) -> p t m", t=2)
                for b in range(HB):
                    gb = b0 + b
                    nc.tensor.matmul(
                        csrs2[:],
                        id8[:],
                        rct[:, b, :, :, :].rearrange("p t r m -> p r t m"),
                        start=(gb == 0),
                        stop=(gb == RPC // 2 - 1),
                        perf_mode=mybir.MatmulPerfMode.DoubleRow,
                        skip_group_check=True,
                    )
                # main term: one DoubleRow matmul per row-pair (pair over t)
                for s in range(QR):
                    qr0 = (i * QR + s) * QR
                    pt = psmain.tile([P, QR, MW], f32, tag="pt")
                    for j in range(2):
                        nc.tensor.matmul(
                            pt[:, 2 * j : 2 * j + 2, :],
                            wst[:],
                            rct[:, 2 * s + j, :, :, :],
                            start=True,
                            stop=True,
                            perf_mode=mybir.MatmulPerfMode.DoubleRow,
                        )
                    with nc.allow_low_precision(reason="f16 staging"):
                        if s % 2 == 0:
                            nc.scalar.activation(
                                Y0[:, qr0 : qr0 + QR, :], pt[:], COPY
                            )
                        else:
                            nc.vector.tensor_copy(Y0[:, qr0 : qr0 + QR, :], pt[:])

            # ---- pre-collective fold: B_pre + payload assembly ----
            csr32 = small.tile([P, 2 * MW], f32)
            nc.scalar.activation(csr32[:], csrs[:], COPY)
            bps = psstat.tile([P, MW], f32, tag="apck")
            nc.tensor.matmul(bps[:], wb_cs[:], csr32[:, 0:MW], start=True, stop=False)
            nc.tensor.matmul(bps[:], wb_rs[:], csr32[:, MW:], start=False, stop=False)
            nc.tensor.matmul(bps[:], wb_dv[:], rdiag[:], start=False, stop=True)
            with nc.allow_low_precision(reason="f16 collective payload"):
                nc.scalar.activation(ccbuf[:, 0:MW], bps[:], COPY)
                nc.vector.tensor_copy(ccbuf[:, MW : 3 * MW], csr32[:])
                dcol = small.tile([P, 1], f32)
                nc.vector.tensor_reduce(
                    dcol[:], rdiag[:], axis=mybir.AxisListType.X, op=add
                )
                nc.vector.tensor_scalar_mul(
                    ccbuf[:, 3 * MW : 3 * MW + 1], dcol[:], smask[:]
                )
                nc.vector.memset(ccbuf[:, 3 * MW + 1 : CCW], 0.0)
            nc.gpsimd.dma_start(cc_in[:], ccbuf[:])
            nc.gpsimd.collective_compute(
                "AllReduce",
                add,
                replica_groups=[list(range(NCORES))],
                ins=[cc_in.opt()],
                outs=[cc_out.opt()],
            )
            nc.gpsimd.dma_start(gbuf[:], cc_out[:])

            # ---- post-collective: A/B/D tables ----
            g_cs = gbuf[:, MW : 2 * MW]
            g_rs = gbuf[:, 2 * MW : 3 * MW]
            B16 = small.tile([P, MW], f16)
            with nc.allow_low_precision(reason="f16 B table"):
                nc.scalar.activation(B16[:], gbuf[:, 0:MW], IDENT, bias=bsum[:])
            stats3 = small.tile([96, MW], f32)
            dvp = psstat.tile([P, MW], f32, tag="apck")
            nc.tensor.matmul(dvp[0:C, :], gk16[:], rdiag[:], start=True, stop=True)
            nc.scalar.activation(stats3[0:C, :], dvp[0:C, :], COPY)
            csp = psstat.tile([P, MW], f32, tag="apck")
            nc.tensor.matmul(csp[0:C, :], gk16[:], g_cs, start=True, stop=True)
            nc.scalar.activation(stats3[32:48, :], csp[0:C, :], COPY)
            rsp = psstat.tile([P, MW], f32, tag="apck")
            nc.tensor.matmul(rsp[0:C, :], gk16[:], g_rs, start=True, stop=True)
            nc.scalar.activation(stats3[64:80, :], rsp[0:C, :], COPY)
            # consts rows: 0-15 dsum | 32-47 tsum
            consts = small.tile([48, 1], f32)
            dsp = psstat.tile([P, MW], f32, tag="apck")
            nc.tensor.matmul(
                dsp[0:C, 0:1], g_all[:], gbuf[:, 3 * MW : 3 * MW + 1],
                start=True, stop=True,
            )
            nc.scalar.activation(consts[0:C, :], dsp[0:C, 0:1], COPY)
            cst2 = small.tile([P, 1], f16)
            with nc.allow_low_precision(reason="f16 total-sum scalar"):
                nc.vector.tensor_reduce(
                    cst2[:], g_cs, axis=mybir.AxisListType.X, op=add
                )
            tsp = psstat.tile([P, MW], f32, tag="apck")
            nc.tensor.matmul(tsp[0:C, 0:1], g_all[:], cst2[:], start=True, stop=True)
            nc.scalar.activation(consts[32:48, :], tsp[0:C, 0:1], COPY)

            ca = small.tile([P, 1], f32)
            cap = psstat.tile([P, MW], f32, tag="apck")
            nc.tensor.matmul(cap[:, 0:1], wca2[:], consts[:], start=True, stop=True)
            nc.scalar.activation(ca[:], cap[:, 0:1], COPY)
            cd = small.tile([P, 1], f32)
            cdp = psstat.tile([P, MW], f32, tag="apck")
            nc.tensor.matmul(cdp[:, 0:1], wcd2[:], consts[:], start=True, stop=True)
            nc.scalar.activation(cd[:], cdp[:, 0:1], COPY)

            A16 = small.tile([P, RPC], f16)
            aps = psstat.tile([P, MW], f32, tag="apck")
            nc.tensor.matmul(aps[:], wa2[:], stats3[:], start=True, stop=True)
            with nc.allow_low_precision(reason="f16 A table"):
                nc.scalar.activation(A16[:], aps[:], IDENT, bias=ca[:])
            # A expanded AXU-wide so pass-2 keeps a packed last dim (DVE 4x)
            A8e = small.tile([P, RPC, AXU], f16)
            with nc.allow_low_precision(reason="f16 A table"):
                nc.vector.tensor_copy(
                    A8e[:],
                    A16.rearrange("p (n x) -> p n x", x=1).broadcast_to(
                        [P, RPC, AXU]
                    ),
                )
            Dm16 = small.tile([P, RPC], f16)
            dps = psstat.tile([P, MW], f32, tag="apck")
            nc.tensor.matmul(dps[:], wd2[:], stats3[:], start=True, stop=True)
            with nc.allow_low_precision(reason="f16 D table"):
                nc.scalar.activation(Dm16[:], dps[:], IDENT, bias=cd[:])
            # A16b = A + sum(bias): per-row bias for the PE-path evictions
            A16b = small.tile([P, RPC], f16)
            with nc.allow_low_precision(reason="f16 A table"):
                nc.scalar.activation(A16b[:], A16[:], IDENT, bias=bsum[:])
            # bt8: fp8 transpose of the bias-free B table, x32 scaled so the
            # small B values stay in fp8e4m3 normal range (ohb carries 1/32)
            bt8 = small.tile([P, 2, P], f8)
            for mb in range(2):
                btp = psstat.tile([P, P], f16, tag="apck")
                nc.tensor.matmul(
                    btp[:],
                    gbuf[:, mb * P : (mb + 1) * P],
                    idt[:],
                    is_transpose=True,
                    start=True,
                    stop=True,
                    skip_group_check=True,
                )
                with nc.allow_low_precision(reason="fp8 B table"):
                    nc.scalar.activation(bt8[:, mb, :], btp[:], COPY, scale=32.0)

            # ---- pass 2: +A (bcast m), +B (bcast r), diag fix, store ----
            # DVE-path blocks: two 2x-mode f16 adds. PE-path blocks: identity
            # copy-matmul + one-hot B matmul into PSUM, Act per-row bias=A
            # eviction. Split keeps the tail under the store-DMA bound.
            for qq in range(NB):
                r0 = qq * BR
                y0b = Y0[:, r0 : r0 + BR, :]
                st = stagep.tile([P, BR, MW], f16, tag="st")
                if qq % 3 == 1:
                    # PE path: 2 q-subblocks of 4 rows
                    pts = []
                    for j in range(2):
                        pt2 = psmain.tile([P, QR, MW], f32, tag="pt")
                        for h in range(2):
                            nc.tensor.matmul(
                                pt2[:, 2 * h : 2 * h + 2, :],
                                idt[:],
                                y0b[:, 4 * j + 2 * h : 4 * j + 2 * h + 2, :],
                                start=True,
                                stop=False,
                                skip_group_check=True,
                            )
                        pts.append(pt2)
                    for j in range(2):
                        for h in range(2):
                            nc.tensor.matmul(
                                pts[j][:, 2 * h : 2 * h + 2, :],
                                bt8[:],
                                ohb[:],
                                start=False,
                                stop=True,
                                perf_mode=mybir.MatmulPerfMode.DoubleRow,
                                skip_group_check=True,
                            )
                    with nc.allow_low_precision(reason="f16 output"):
                        for r in range(BR):
                            nc.scalar.activation(
                                st[:, r, :],
                                pts[r // 4][:, r % 4, :],
                                IDENT,
                                bias=A16b[:, r0 + r : r0 + r + 1],
                            )
                else:
                    y0b4 = y0b.rearrange("p n (u x) -> p n u x", x=AXU)
                    with nc.allow_low_precision(reason="f16 output"):
                        nc.vector.tensor_tensor(
                            y0b4[:],
                            y0b4[:],
                            A8e[:, r0 : r0 + BR, :]
                            .rearrange("p n (u x) -> p n u x", u=1)
                            .broadcast_to([P, BR, MW // AXU, AXU]),
                            op=add,
                        )
                        nc.vector.tensor_tensor(
                            st[:],
                            y0b[:],
                            B16.rearrange("p (n m) -> p n m", n=1).broadcast_to(
                                [P, BR, MW]
                            ),
                            op=add,
                        )
                with nc.allow_low_precision(reason="f16 output"):
                    # diagonal fix-up (valid only on g=k partitions; Dm16
                    # is masked to zero elsewhere)
                    stflat = st.rearrange("p n m -> p (n m)")
                    nc.gpsimd.tensor_tensor(
                        stflat[:, r0 : r0 + (BR - 1) * (MW + 1) + 1 : MW + 1],
                        stflat[:, r0 : r0 + (BR - 1) * (MW + 1) + 1 : MW + 1],
                        Dm16[:, r0 : r0 + BR],
                        op=add,
                    )
                nc.sync.dma_start(y_d[:, r0 : r0 + BR, :], st[:])

    nc.compile()
    return nc


def _host_prep(X, weights, bias):
    """Pack panels + fold weights into per-core input maps."""
    import ml_dtypes

    W = weights.astype(np.float32)
    iN = np.float32(1.0 / N)
    iN2 = np.float32(1.0 / (N * N))
    bias_sum = np.float32(bias.astype(np.float64).sum())

    Xr = np.ascontiguousarray(X[0])  # [C, N, N] fp32
    Rp = (
        Xr.reshape(C, NCORES, RPC, G, MW)
        .transpose(1, 3, 0, 2, 4)
        .reshape(NCORES, P, RPC, MW)
        .astype(ml_dtypes.float8_e4m3)
    )
    XT = np.ascontiguousarray(Xr.transpose(0, 2, 1))
    Cp = (
        XT.reshape(C, NCORES, RPC, G, MW)
        .transpose(1, 3, 0, 2, 4)
        .reshape(NCORES, P, RPC, MW)
        .astype(ml_dtypes.float8_e4m3)
    )
    # interleave: rc8[k, p, b, t, r2, m], row = 2b + r2; t=0 rows, t=1 cols
    rc8 = np.ascontiguousarray(
        np.stack(
            [
                Rp.reshape(NCORES, P, RPC // 2, 2, MW),
                Cp.reshape(NCORES, P, RPC // 2, 2, MW),
            ],
            axis=3,
        )
    )

    def blockdiag(w, dtype):
        out = np.zeros((P, P), dtype=dtype)
        for g in range(G):
            out[g * C : (g + 1) * C, g * C : (g + 1) * C] = w
        return out

    wst = np.stack(
        [
            blockdiag(W[8], np.float32),
            blockdiag(W[6], np.float32),
        ],
        axis=1,
    ).astype(ml_dtypes.float8_e4m3)  # [128, 2, 128]

    id8 = np.stack([np.eye(P, dtype=np.float32)] * 2, axis=1).astype(
        ml_dtypes.float8_e4m3
    )  # [128, 2, 128] stacked identities for the pair-sum matmuls
    idt = np.eye(P, dtype=np.float16)
    # one-hot B pattern (x 1/32; bt8 is x32): ohb[k, t, r, m] = (m == 128t+k)/32
    ohb = np.zeros((P, 2, 2, N // G), np.float32)
    for t in range(2):
        for k in range(P):
            ohb[k, t, :, t * P + k] = 1.0 / 32.0
    ohb = ohb.astype(ml_dtypes.float8_e4m3)
    wb_cs = blockdiag(W[10] * iN, np.float32)
    wb_rs = blockdiag(W[13] * iN, np.float32)
    g_all = np.tile(np.eye(C, dtype=np.float32), (G, 1))  # [128, 16]

    def rep(w):  # [16,16] -> [16, 128]
        return np.tile(w.astype(np.float32), (1, G))

    in_maps = []
    for k in range(NCORES):
        rowmask = np.repeat((np.arange(G) == k).astype(np.float32), C)  # [128]
        gk = g_all * rowmask[:, None]
        wb_dv = blockdiag(W[9], np.float32) * rowmask[:, None]
        wad = np.zeros((2, 96, P), np.float32)
        wad[0, 0:C] = rep(W[5])
        wad[0, 32:48] = rep(W[7] * iN)
        wad[0, 64:80] = rep(W[12] * iN)
        wad[1, 0:C] = rep(W[0]) * rowmask[None, :]
        wad[1, 32:48] = rep(W[1] * iN) * rowmask[None, :]
        wad[1, 64:80] = rep(W[3] * iN) * rowmask[None, :]
        wcc = np.zeros((2, 48, P), np.float32)
        wcc[0, 0:C] = rep(W[11] * iN)
        wcc[0, 32:48] = rep(W[14] * iN2)
        wcc[1, 0:C] = rep(W[2] * iN) * rowmask[None, :]
        wcc[1, 32:48] = rep(W[4] * iN2) * rowmask[None, :]
        in_maps.append(
            {
                "rc8": rc8[k],
                "wst": wst,
                "id8": id8,
                "idt": idt,
                "ohb": ohb,
                "wb_cs": wb_cs,
                "wb_dv": wb_dv.astype(np.float16),
                "wb_rs": wb_rs,
                "gk16": gk.astype(np.float16),
                "g_all": g_all.astype(np.float16),
                "wad": wad,
                "wcc": wcc,
                "smask": rowmask[:, None].copy(),
                "bsum": np.full((P, 1), bias_sum, np.float32),
            }
        )
    return in_maps


def kernel(X, weights, bias):
    if "nc" not in _CACHED:
        _CACHED["nc"] = _build_program()
    nc = _CACHED["nc"]

    trace = bool(os.environ.get("BASS_TRACE"))
    if trace:
        _install_trace_hook()

    in_maps = _host_prep(np.asarray(X), np.asarray(weights), np.asarray(bias))
    res = bass_utils.run_bass_kernel_spmd(
        nc, in_maps, core_ids=list(range(NCORES)), trace=trace
    )
    LAST_RUN_INFO.clear()
    LAST_RUN_INFO.update(
        exec_time_ns=res.exec_time_ns,
        mean_exec_time_ns=res.mean_exec_time_ns,
        trace=res.instructions_and_trace[1] if res.instructions_and_trace else None,
    )

    Yp = np.stack([np.asarray(res.results[k]["y"]) for k in range(NCORES)])
    Y = (
        Yp.astype(np.float32)
        .reshape(NCORES, G, C, RPC, MW)
        .transpose(2, 0, 3, 1, 4)
        .reshape(1, C, N, N)
    )
    return Y


# revision 44
# speedup vs baseline: 1.4375x; 1.0319x over previous
"""Equivariant layer block (order-2, 15-basis) on 8 Trainium2 NeuronCores.

Decomposition (indices: c in-channel, o out-channel, n/m spatial, N=2048):
  Y[o,n,m] = sum_c X[c,n,m] W8[c,o] + X[c,m,n] W6[c,o]
           + A[o,n] + B[o,m] + D[o,n] delta[n,m]
with (raw sums; /N factors folded into host-side weights; i = ref basis index)
  A[o,n] = dv.W5 + csum.W7/N + rsum.W12/N + dsum.W11/N + tsum.W14/N^2
  B[o,m] = dv.W9 + csum.W10/N + rsum.W13/N + sum(bias)
  D[o,n] = dv.W0 + csum.W1/N + rsum.W3/N + dsum.W2/N + tsum.W4/N^2

v4 design. Core k owns output rows I_k=[256k,256k+256). Both spatial panels
stream through a 4-chunk fp8 SBUF ring rc[(g,c), b, t, r2, m] (t=0 rows,
t=1 cols); per chunk the PE does the fused DoubleRow main matmul (identity +
transpose terms via the two k-tiles) plus identity pair-sum matmuls that
accumulate partial column sums (from the row panel) AND partial row sums
(from the column panel) in one persistent PSUM bank. The main term is
evicted table-free into a full f16 staging tile Y0 (Act/DVE alternating
plain copies), so the whole load phase is DMA-bound and the PE never
switches stationary weights mid-phase.

One f16 AllReduce of [128, 770] (pre-folded B table | csum partials | rsum
partials | masked diag column) fires as soon as the last chunk's pair-sums
land. Post-collective, a short matmul chain builds the A/B/D tables, then
pass 2 walks Y0 in 8-row blocks: two DVE 4x-mode broadcast adds (+A along
m, +B along r), a tiny Pool diag fix-up, and pipelined f16 stores on the
sync/gpsimd queues. sum(bias) is folded into B on device.
"""

import os
import numpy as np

import concourse.bacc as bacc
import concourse.tile as tile
import concourse.mybir as mybir
from concourse import bass_utils

N = 2048
C = 16
NCORES = 8
RPC = N // NCORES  # 256 rows per core
G = 8  # m-groups
MW = N // G  # 256
P = 128
CHUNK = 32  # rows per DMA chunk
NCHUNK = RPC // CHUNK  # 8
HB = CHUNK // 2  # row-pairs per chunk (16)
QR = 4  # rows per main-loop quarter-chunk
NQ = RPC // QR  # 64
BR = 8  # rows per pass-2 block
NB = RPC // BR  # 32
AXU = 8  # inner expansion of the A table (packed last dim for DVE 2x)
CCW = 2 * 256 + 256 + 2  # AllReduce payload cols: B_pre|csum|rsum|dcol|pad
f16 = mybir.dt.float16
f32 = mybir.dt.float32
f8 = mybir.dt.float8e4

LAST_RUN_INFO = {}
_CACHED = {}


def _install_trace_hook():
    """Best-effort NTFF hook injection (used only when BASS_TRACE is set)."""
    try:
        import sys, types

        if "antenv.axon_hooks" in sys.modules:
            return
        mod = types.ModuleType("antenv.axon_hooks")
        state = {}
        mod.set_axon_ntff_profile_hook = lambda h: state.update(h=h)
        mod.get_axon_ntff_profile_hook = lambda: state.get("h")
        sys.modules["antenv.axon_hooks"] = mod
        import antenv

        antenv.axon_hooks = mod
        from trn_agent_boot.trn_boot import _ntff_profile_via_ctypes

        mod.set_axon_ntff_profile_hook(
            _ntff_profile_via_ctypes("/opt/axon/libaxon_pjrt.so")
        )
    except Exception:
        pass


def _build_program():
    nc = bacc.Bacc("TRN2", target_bir_lowering=False, debug=False, num_devices=NCORES)

    # interleaved panel: rc_d[p, b, t, r2, m], row = 2b+r2, t=0 rows / t=1 cols
    rc_d = nc.dram_tensor("rc8", [P, RPC // 2, 2, 2, MW], f8, kind="ExternalInput").ap()
    wst_d = nc.dram_tensor("wst", [P, 2, P], f8, kind="ExternalInput").ap()
    id8_d = nc.dram_tensor("id8", [P, 2, P], f8, kind="ExternalInput").ap()
    ohb_d = nc.dram_tensor("ohb", [P, 2, 2, MW], f8, kind="ExternalInput").ap()
    idt_d = nc.dram_tensor("idt", [P, P], f16, kind="ExternalInput").ap()
    wbcs_d = nc.dram_tensor("wb_cs", [P, P], f32, kind="ExternalInput").ap()
    wbdv_d = nc.dram_tensor("wb_dv", [P, P], f16, kind="ExternalInput").ap()
    wbrs_d = nc.dram_tensor("wb_rs", [P, P], f32, kind="ExternalInput").ap()
    gk16_d = nc.dram_tensor("gk16", [P, C], f16, kind="ExternalInput").ap()
    gall_d = nc.dram_tensor("g_all", [P, C], f16, kind="ExternalInput").ap()
    wad_d = nc.dram_tensor("wad", [2, 96, P], f32, kind="ExternalInput").ap()
    wcc_d = nc.dram_tensor("wcc", [2, 48, P], f32, kind="ExternalInput").ap()
    smask_d = nc.dram_tensor("smask", [P, 1], f32, kind="ExternalInput").ap()
    bsum_d = nc.dram_tensor("bsum", [P, 1], f32, kind="ExternalInput").ap()

    y_d = nc.dram_tensor("y", [P, RPC, MW], f16, kind="ExternalOutput").ap()

    add = mybir.AluOpType.add
    COPY = mybir.ActivationFunctionType.Copy
    IDENT = mybir.ActivationFunctionType.Identity

    with tile.TileContext(nc) as tc:
        with (
            tc.tile_pool(name="small", bufs=1) as small,
            tc.tile_pool(name="rcring", bufs=3) as rcring,
            tc.tile_pool(name="y0p", bufs=1) as y0p,
            tc.tile_pool(name="pscr", bufs=1, space="PSUM") as pscr,
            tc.tile_pool(name="psstat", bufs=1, space="PSUM") as psstat,
            tc.tile_pool(name="psmain", bufs=3, space="PSUM") as psmain,
            tc.tile_pool(name="dram", bufs=1, space="DRAM") as dram,
        ):
            # ---- kick the first input chunks before the small weights ----
            rcts = []
            for i in range(NCHUNK):
                rct = rcring.tile([P, HB, 2, 2, MW], f8, tag="rc", name=f"rc{i}")
                rcts.append(rct)
            for i in range(2):
                nc.sync.dma_start(
                    rcts[i][:], rc_d[:, i * HB : (i + 1) * HB, :, :, :]
                )

            # ---- constant / weight loads ----
            wst = small.tile([P, 2, P], f8)
            id8 = small.tile([P, 2, P], f8)
            ohb = small.tile([P, 2, 2, MW], f8)
            idt = small.tile([P, P], f16)
            wb_cs = small.tile([P, P], f32)
            wb_dv = small.tile([P, P], f16)
            wb_rs = small.tile([P, P], f32)
            gk16 = small.tile([P, C], f16)
            g_all = small.tile([P, C], f16)
            smask = small.tile([P, 1], f32)
            bsum = small.tile([P, 1], f32)
            for t, d in [
                (wst, wst_d),
                (id8, id8_d),
                (ohb, ohb_d),
                (idt, idt_d),
                (wb_cs, wbcs_d),
                (wb_dv, wbdv_d),
                (wb_rs, wbrs_d),
                (gk16, gk16_d),
                (g_all, gall_d),
                (smask, smask_d),
                (bsum, bsum_d),
            ]:
                nc.sync.dma_start(t[:], d[:])
            wa3 = small.tile([96, P], f32)
            wd3 = small.tile([96, P], f32)
            wca2 = small.tile([48, P], f32)
            wcd2 = small.tile([48, P], f32)
            nc.sync.dma_start(wa3[:], wad_d[0])
            nc.sync.dma_start(wd3[:], wad_d[1])
            nc.sync.dma_start(wca2[:], wcc_d[0])
            nc.sync.dma_start(wcd2[:], wcc_d[1])

            Y0 = y0p.tile([P, RPC, MW], f16)  # staged main term (pre-tables)
            rdiag = small.tile([P, RPC], f16)  # diag per group (g=k rows valid)
            # csum/rsum pair-sum accumulators share one PSUM bank: [cs | rs]
            csrs = pscr.tile([P, 2 * MW], f32)
            ccbuf = small.tile([P, CCW], f16)
            gbuf = small.tile([P, CCW], f16)
            cc_in = dram.tile([P, CCW], f16)
            cc_out = dram.tile([P, CCW], f16)

            # ---- load phase: stream chunks, stats + main matmuls + evict ----
            for i in range(NCHUNK):
                r0 = i * CHUNK
                b0 = i * HB
                rct = rcts[i]
                if i >= 2:
                    nc.sync.dma_start(rct[:], rc_d[:, b0 : b0 + HB, :, :, :])
                # diag extract: row r=2b'+r2 at flat offset 1026*b' + 257*r2
                rcflat = rct.rearrange("p b t r m -> p (b t r m)")
                for r2i in range(2):
                    nc.scalar.activation(
                        rdiag[:, r0 + r2i : r0 + CHUNK : 2],
                        rcflat[
                            :,
                            257 * r2i + r0 : 257 * r2i + r0 + (HB - 1) * 1026 + 1 : 1026,
                        ],
                        COPY,
                    )
                # pair-sum stats on PE: one DoubleRow matmul per row-pair
                # covers BOTH panels (pair over r2; free dims [t, m]):
                # csrs[(g,c), t*MW+m] += rct[.., b, t, 0, m] + rct[.., b, t, 1, m]
                csrs2 = csrs.rearrange("p (t m) -> p t m", t=2)
                for b in range(HB):
                    gb = b0 + b
                    nc.tensor.matmul(
                        csrs2[:],
                        id8[:],
                        rct[:, b, :, :, :].rearrange("p t r m -> p r t m"),
                        start=(gb == 0),
                        stop=(gb == RPC // 2 - 1),
                        perf_mode=mybir.MatmulPerfMode.DoubleRow,
                        skip_group_check=True,
                    )
                # main term: one DoubleRow matmul per row-pair (pair over t)
                for s in range(CHUNK // QR):
                    qr0 = r0 + s * QR
                    pt = psmain.tile([P, QR, MW], f32, tag="pt")
                    for j in range(2):
                        nc.tensor.matmul(
                            pt[:, 2 * j : 2 * j + 2, :],
                            wst[:],
                            rct[:, 2 * s + j, :, :, :],
                            start=True,
                            stop=True,
                            perf_mode=mybir.MatmulPerfMode.DoubleRow,
                        )
                    with nc.allow_low_precision(reason="f16 staging"):
                        if s % 2 == 0:
                            nc.scalar.activation(
                                Y0[:, qr0 : qr0 + QR, :], pt[:], COPY
                            )
                        else:
                            nc.vector.tensor_copy(Y0[:, qr0 : qr0 + QR, :], pt[:])

            # ---- pre-collective fold: B_pre + payload assembly ----
            csr32 = small.tile([P, 2 * MW], f32)
            nc.scalar.activation(csr32[:], csrs[:], COPY)
            bps = psstat.tile([P, MW], f32, tag="apck")
            nc.tensor.matmul(bps[:], wb_cs[:], csr32[:, 0:MW], start=True, stop=False)
            nc.tensor.matmul(bps[:], wb_rs[:], csr32[:, MW:], start=False, stop=False)
            nc.tensor.matmul(bps[:], wb_dv[:], rdiag[:], start=False, stop=True)
            with nc.allow_low_precision(reason="f16 collective payload"):
                nc.scalar.activation(ccbuf[:, 0:MW], bps[:], COPY)
                nc.vector.tensor_copy(ccbuf[:, MW : 3 * MW], csr32[:])
                dcol = small.tile([P, 1], f32)
                nc.vector.tensor_reduce(
                    dcol[:], rdiag[:], axis=mybir.AxisListType.X, op=add
                )
                nc.vector.tensor_scalar_mul(
                    ccbuf[:, 3 * MW : 3 * MW + 1], dcol[:], smask[:]
                )
                nc.vector.memset(ccbuf[:, 3 * MW + 1 : CCW], 0.0)
            nc.gpsimd.dma_start(cc_in[:], ccbuf[:])
            # local dv stats fold overlaps the collective
            stats3 = small.tile([96, MW], f32)
            dvp = psstat.tile([P, MW], f32, tag="apck")
            nc.tensor.matmul(dvp[0:C, :], gk16[:], rdiag[:], start=True, stop=True)
            nc.scalar.activation(stats3[0:C, :], dvp[0:C, :], COPY)
            nc.gpsimd.collective_compute(
                "AllReduce",
                add,
                replica_groups=[list(range(NCORES))],
                ins=[cc_in.opt()],
                outs=[cc_out.opt()],
            )
            nc.gpsimd.dma_start(gbuf[:], cc_out[:])

            # ---- post-collective: A/B/D tables ----
            # stats3 rows: 0 dv | 16 csum | 32 rsum | 48 dsum-bcast | 64 tsum-b
            g_cs = gbuf[:, MW : 2 * MW]
            g_rs = gbuf[:, 2 * MW : 3 * MW]
            B16 = small.tile([P, MW], f16)
            with nc.allow_low_precision(reason="f16 B table"):
                nc.scalar.activation(B16[:], gbuf[:, 0:MW], IDENT, bias=bsum[:])
            csp = psstat.tile([P, MW], f32, tag="apck")
            nc.tensor.matmul(csp[0:C, :], gk16[:], g_cs, start=True, stop=True)
            nc.scalar.activation(stats3[32:48, :], csp[0:C, :], COPY)
            rsp = psstat.tile([P, MW], f32, tag="apck")
            nc.tensor.matmul(rsp[0:C, :], gk16[:], g_rs, start=True, stop=True)
            nc.scalar.activation(stats3[64:80, :], rsp[0:C, :], COPY)
            # stats2: dsum/tsum broadcast rows (32-aligned partition bases)
            stats2 = small.tile([48, MW], f32)
            dsp = psstat.tile([P, MW], f32, tag="apck")
            nc.tensor.matmul(
                dsp[0:C, 0:1], g_all[:], gbuf[:, 3 * MW : 3 * MW + 1],
                start=True, stop=True,
            )
            nc.vector.tensor_copy(
                stats2[0:C, :], dsp[0:C, 0:1].broadcast_to([C, MW])
            )
            cst2 = small.tile([P, 1], f16)
            with nc.allow_low_precision(reason="f16 total-sum scalar"):
                nc.vector.tensor_reduce(
                    cst2[:], g_cs, axis=mybir.AxisListType.X, op=add
                )
            tsp = psstat.tile([P, MW], f32, tag="apck")
            nc.tensor.matmul(tsp[0:C, 0:1], g_all[:], cst2[:], start=True, stop=True)
            nc.vector.tensor_copy(
                stats2[32:48, :], tsp[0:C, 0:1].broadcast_to([C, MW])
            )

            A16 = small.tile([P, RPC], f16)
            aps = psstat.tile([P, MW], f32, tag="apck")
            nc.tensor.matmul(aps[:], wa3[:], stats3[:], start=True, stop=False)
            nc.tensor.matmul(aps[:], wca2[:], stats2[:], start=False, stop=True)
            with nc.allow_low_precision(reason="f16 A table"):
                nc.scalar.activation(A16[:], aps[:], COPY)
            # A expanded AXU-wide so pass-2 keeps a packed last dim (DVE 4x)
            A8e = small.tile([P, RPC, AXU], f16)
            with nc.allow_low_precision(reason="f16 A table"):
                nc.vector.tensor_copy(
                    A8e[:],
                    A16.rearrange("p (n x) -> p n x", x=1).broadcast_to(
                        [P, RPC, AXU]
                    ),
                )
            Dm16 = small.tile([P, RPC], f16)
            dps = psstat.tile([P, MW], f32, tag="apck")
            nc.tensor.matmul(dps[:], wd3[:], stats3[:], start=True, stop=False)
            nc.tensor.matmul(dps[:], wcd2[:], stats2[:], start=False, stop=True)
            with nc.allow_low_precision(reason="f16 D table"):
                nc.scalar.activation(Dm16[:], dps[:], COPY)
            # A16b = A + sum(bias): per-row bias for the PE-path evictions
            A16b = small.tile([P, RPC], f16)
            with nc.allow_low_precision(reason="f16 A table"):
                nc.scalar.activation(A16b[:], A16[:], IDENT, bias=bsum[:])
            # bt8: fp8 transpose of the bias-free B table, x32 scaled so the
            # small B values stay in fp8e4m3 normal range (ohb carries 1/32)
            bt8 = small.tile([P, 2, P], f8)
            for mb in range(2):
                btp = psstat.tile([P, P], f16, tag="apck")
                nc.tensor.matmul(
                    btp[:],
                    gbuf[:, mb * P : (mb + 1) * P],
                    idt[:],
                    is_transpose=True,
                    start=True,
                    stop=True,
                    skip_group_check=True,
                )
                with nc.allow_low_precision(reason="fp8 B table"):
                    nc.scalar.activation(bt8[:, mb, :], btp[:], COPY, scale=32.0)

            # ---- diag fix-up: one strided add over ALL of Y0 (row r diag
            # sits at flat r*257 within [r, m']; Dm16 is zero off g=k) ----
            y0flat = Y0.rearrange("p n m -> p (n m)")
            with nc.allow_low_precision(reason="f16 output"):
                nc.gpsimd.tensor_tensor(
                    y0flat[:, 0 : (RPC - 1) * (MW + 1) + 1 : MW + 1],
                    y0flat[:, 0 : (RPC - 1) * (MW + 1) + 1 : MW + 1],
                    Dm16[:],
                    op=add,
                )

            # ---- pass 2: +A (bcast m), +B (bcast r) in place on Y0, store --
            # DVE-path blocks: two 2x-mode f16 adds. PE-path blocks: identity
            # copy-matmul + one-hot B matmul into PSUM, Act per-row bias=A
            # eviction. Split keeps the tail under the store-DMA bound.
            pe_path = {qq for qq in range(NB) if qq % 5 in (1, 3)}
            for qq in range(NB):
                r0 = qq * BR
                y0b = Y0[:, r0 : r0 + BR, :]
                if qq in pe_path:
                    # PE path: 2 q-subblocks of 4 rows
                    pts = []
                    for j in range(2):
                        pt2 = psmain.tile([P, QR, MW], f32, tag="pt")
                        for h in range(2):
                            nc.tensor.matmul(
                                pt2[:, 2 * h : 2 * h + 2, :],
                                idt[:],
                                y0b[:, 4 * j + 2 * h : 4 * j + 2 * h + 2, :],
                                start=True,
                                stop=False,
                                skip_group_check=True,
                            )
                        pts.append(pt2)
                    for j in range(2):
                        for h in range(2):
                            nc.tensor.matmul(
                                pts[j][:, 2 * h : 2 * h + 2, :],
                                bt8[:],
                                ohb[:],
                                start=False,
                                stop=True,
                                perf_mode=mybir.MatmulPerfMode.DoubleRow,
                                skip_group_check=True,
                            )
                    with nc.allow_low_precision(reason="f16 output"):
                        for r in range(BR):
                            nc.scalar.activation(
                                y0b[:, r, :],
                                pts[r // 4][:, r % 4, :],
                                IDENT,
                                bias=A16b[:, r0 + r : r0 + r + 1],
                            )
                else:
                    y0b4 = y0b.rearrange("p n (u x) -> p n u x", x=AXU)
                    with nc.allow_low_precision(reason="f16 output"):
                        nc.vector.tensor_tensor(
                            y0b4[:],
                            y0b4[:],
                            A8e[:, r0 : r0 + BR, :]
                            .rearrange("p n (u x) -> p n u x", u=1)
                            .broadcast_to([P, BR, MW // AXU, AXU]),
                            op=add,
                        )
                        nc.vector.tensor_tensor(
                            y0b[:],
                            y0b[:],
                            B16.rearrange("p (n m) -> p n m", n=1).broadcast_to(
                                [P, BR, MW]
                            ),
                            op=add,
                        )
                nc.sync.dma_start(y_d[:, r0 : r0 + BR, :], y0b[:])

    nc.compile()
    return nc


def _host_prep(X, weights, bias):
    """Pack panels + fold weights into per-core input maps."""
    import ml_dtypes

    W = weights.astype(np.float32)
    iN = np.float32(1.0 / N)
    iN2 = np.float32(1.0 / (N * N))
    bias_sum = np.float32(bias.astype(np.float64).sum())

    Xr = np.ascontiguousarray(X[0])  # [C, N, N] fp32
    Rp = (
        Xr.reshape(C, NCORES, RPC, G, MW)
        .transpose(1, 3, 0, 2, 4)
        .reshape(NCORES, P, RPC, MW)
        .astype(ml_dtypes.float8_e4m3)
    )
    XT = np.ascontiguousarray(Xr.transpose(0, 2, 1))
    Cp = (
        XT.reshape(C, NCORES, RPC, G, MW)
        .transpose(1, 3, 0, 2, 4)
        .reshape(NCORES, P, RPC, MW)
        .astype(ml_dtypes.float8_e4m3)
    )
    # interleave: rc8[k, p, b, t, r2, m], row = 2b + r2; t=0 rows, t=1 cols
    rc8 = np.ascontiguousarray(
        np.stack(
            [
                Rp.reshape(NCORES, P, RPC // 2, 2, MW),
                Cp.reshape(NCORES, P, RPC // 2, 2, MW),
            ],
            axis=3,
        )
    )

    def blockdiag(w, dtype):
        out = np.zeros((P, P), dtype=dtype)
        for g in range(G):
            out[g * C : (g + 1) * C, g * C : (g + 1) * C] = w
        return out

    wst = np.stack(
        [
            blockdiag(W[8], np.float32),
            blockdiag(W[6], np.float32),
        ],
        axis=1,
    ).astype(ml_dtypes.float8_e4m3)  # [128, 2, 128]

    id8 = np.stack([np.eye(P, dtype=np.float32)] * 2, axis=1).astype(
        ml_dtypes.float8_e4m3
    )  # [128, 2, 128] stacked identities for the pair-sum matmuls
    idt = np.eye(P, dtype=np.float16)
    # one-hot B pattern (x 1/32; bt8 is x32): ohb[k, t, r, m] = (m == 128t+k)/32
    ohb = np.zeros((P, 2, 2, N // G), np.float32)
    for t in range(2):
        for k in range(P):
            ohb[k, t, :, t * P + k] = 1.0 / 32.0
    ohb = ohb.astype(ml_dtypes.float8_e4m3)
    wb_cs = blockdiag(W[10] * iN, np.float32)
    wb_rs = blockdiag(W[13] * iN, np.float32)
    g_all = np.tile(np.eye(C, dtype=np.float32), (G, 1))  # [128, 16]

    def rep(w):  # [16,16] -> [16, 128]
        return np.tile(w.astype(np.float32), (1, G))

    in_maps = []
    for k in range(NCORES):
        rowmask = np.repeat((np.arange(G) == k).astype(np.float32), C)  # [128]
        gk = g_all * rowmask[:, None]
        wb_dv = blockdiag(W[9], np.float32) * rowmask[:, None]
        # stats3 rows: 0 dv | 32 csum | 64 rsum; stats2: 0 dsum-b | 32 tsum-b
        wad = np.zeros((2, 96, P), np.float32)
        wad[0, 0:C] = rep(W[5])
        wad[0, 32:48] = rep(W[7] * iN)
        wad[0, 64:80] = rep(W[12] * iN)
        wad[1, 0:C] = rep(W[0]) * rowmask[None, :]
        wad[1, 32:48] = rep(W[1] * iN) * rowmask[None, :]
        wad[1, 64:80] = rep(W[3] * iN) * rowmask[None, :]
        wcc = np.zeros((2, 48, P), np.float32)
        wcc[0, 0:C] = rep(W[11] * iN)
        wcc[0, 32:48] = rep(W[14] * iN2)
        wcc[1, 0:C] = rep(W[2] * iN) * rowmask[None, :]
        wcc[1, 32:48] = rep(W[4] * iN2) * rowmask[None, :]
        in_maps.append(
            {
                "rc8": rc8[k],
                "wst": wst,
                "id8": id8,
                "idt": idt,
                "ohb": ohb,
                "wb_cs": wb_cs,
                "wb_dv": wb_dv.astype(np.float16),
                "wb_rs": wb_rs,
                "gk16": gk.astype(np.float16),
                "g_all": g_all.astype(np.float16),
                "wad": wad,
                "wcc": wcc,
                "smask": rowmask[:, None].copy(),
                "bsum": np.full((P, 1), bias_sum, np.float32),
            }
        )
    return in_maps


def kernel(X, weights, bias):
    if "nc" not in _CACHED:
        _CACHED["nc"] = _build_program()
    nc = _CACHED["nc"]

    trace = bool(os.environ.get("BASS_TRACE"))
    if trace:
        _install_trace_hook()

    in_maps = _host_prep(np.asarray(X), np.asarray(weights), np.asarray(bias))
    res = bass_utils.run_bass_kernel_spmd(
        nc, in_maps, core_ids=list(range(NCORES)), trace=trace
    )
    LAST_RUN_INFO.clear()
    LAST_RUN_INFO.update(
        exec_time_ns=res.exec_time_ns,
        mean_exec_time_ns=res.mean_exec_time_ns,
        trace=res.instructions_and_trace[1] if res.instructions_and_trace else None,
    )

    Yp = np.stack([np.asarray(res.results[k]["y"]) for k in range(NCORES)])
    Y = (
        Yp.astype(np.float32)
        .reshape(NCORES, G, C, RPC, MW)
        .transpose(2, 0, 3, 1, 4)
        .reshape(1, C, N, N)
    )
    return Y
